# revision 4
# baseline (speedup 1.0000x reference)
"""Trainium2 Bass kernel for CMELossAngularProfileMSE_V2.

Strategy (pure data parallel over batch, 8 NeuronCores):
  - Shard B=128 samples -> 16 per core.
  - Per core, per sample: DMA a [128, 5760] tile in 2 half-sample
    chunks (r-major within partition: partition p holds r in
    [16p, 16p+16), free dim = 16*360 contiguous). H0 chunks issue on
    the sync queue, H1 chunks on the scalar queue so two HWDGE
    descriptor generators run in parallel.
  - The 16->1 q-slice fold is split across engines so no engine comes
    close to the ~6.3us/sample wire cadence (DVE alone at ~6.4us/sample
    was the old limiter and made the pipeline fragile):
      DVE:    H0 tree 1440+720+360 and H1 tail 720+360  (~4.5us)
      GPSIMD: H1 head 1440                              (~3.3us)
      PE:     2 single-row matmuls (ones stationary) accumulating the
              two surviving 360-slices into PSUM row b  (~1.7us)
    Rows are independent accumulation groups (start/stop per sample),
    so there is no cross-sample PSUM serialization.
  - Host precomputes T' = R*T and w' = w/R^2 (exact power-of-two
    scalings of the Gaussian target / distance weight derived from
    theta_min/theta_max), so the device epilogue is just
    sum_theta((S - T')^2 * w') per sample -> out [16, 1], on DVE.
  - Host: loss = sum(all per-sample sums) / (360 * 128).
"""
import numpy as np

import concourse.bacc as bacc
import concourse.tile as tile
from concourse import mybir
from concourse.bass_utils import run_bass_kernel_spmd

F32 = mybir.dt.float32
ADD = mybir.AluOpType.add

N_CORES = 8
B = 128            # full batch
BS = B // N_CORES  # samples per core (16)
R = 2048
TH = 360
Q = 16             # r-slices per partition (2048 = 128 * 16)
SIGMA = 10.0
ALPHA_WEIGHT = 2.0
LAMBDA_ANG = 1.0

H = (Q // 2) * TH  # half-sample width (2880)


def _build_nc():
    nc = bacc.Bacc("TRN2", target_bir_lowering=False, debug=False)
    x = nc.dram_tensor("x", [BS, 128, Q * TH], F32, kind="ExternalInput").ap()
    tw = nc.dram_tensor("tw", [2, BS, TH], F32, kind="ExternalInput").ap()
    out = nc.dram_tensor("out", [BS, 1], F32, kind="ExternalOutput").ap()

    from contextlib import ExitStack
    with tile.TileContext(nc) as tc, ExitStack() as ctx:
        consts = ctx.enter_context(tc.tile_pool(name="consts", bufs=1))
        inp = ctx.enter_context(tc.tile_pool(name="inp", bufs=7))
        psum = ctx.enter_context(tc.tile_pool(name="psum", bufs=1, space="PSUM"))
        small = ctx.enter_context(tc.tile_pool(name="small", bufs=1))

        # one-hot weight matrices: O[:, b, j] = 1 if j == b else 0
        # (PSUM out base partition must be 0/32/64, so per-row matmuls
        # need the one-hot trick; built on gpsimd, idle at startup)
        O = consts.tile([128, BS, BS], F32)
        nc.gpsimd.memset(O[:], 0.0)
        for b in range(BS):
            nc.gpsimd.memset(O[:, b, b:b + 1], 1.0)

        # tw holds T' = R*T and w' = w/R^2 (exact power-of-two scalings),
        # so the raw PSUM sums S feed the epilogue directly. Issued on
        # the scalar queue ahead of the H1 stream (one dispatch slot).
        t16w16 = small.tile([BS, 2, TH], F32)
        t16 = t16w16[:, 0, :]
        w16 = t16w16[:, 1, :]
        nc.scalar.dma_start(t16w16[:], tw.rearrange("two b t -> b two t"))

        ps = psum.tile([BS, TH], F32)
        for b in range(BS):
            xt = inp.tile([128, Q * TH], F32)
            if b < BS - 1:
                # steady state: two half-sample chunks on two queues
                nc.sync.dma_start(xt[:, 0:H], x[b][:, 0:H])
                nc.scalar.dma_start(xt[:, H:2 * H], x[b][:, H:2 * H])
                # H0 fold tree on DVE
                nc.vector.tensor_add(xt[:, 0:1440], xt[:, 0:1440],
                                     xt[:, 1440:2880])
                # H1 head fold on GPSIMD (its only steady-state job)
                nc.gpsimd.tensor_tensor(xt[:, 2880:4320], xt[:, 2880:4320],
                                        xt[:, 4320:5760], ADD)
                nc.vector.tensor_add(xt[:, 0:720], xt[:, 0:720],
                                     xt[:, 720:1440])
                nc.vector.tensor_add(xt[:, 0:360], xt[:, 0:360],
                                     xt[:, 360:720])
                nc.tensor.matmul(ps[:], O[:, b, :], xt[:, 0:360],
                                 start=(b == 0), stop=False)
                # H1 tail folds on DVE once GPSIMD's head add lands
                nc.vector.tensor_add(xt[:, 2880:3600], xt[:, 2880:3600],
                                     xt[:, 3600:4320])
                nc.vector.tensor_add(xt[:, 2880:3240], xt[:, 2880:3240],
                                     xt[:, 3240:3600])
                nc.tensor.matmul(ps[:], O[:, b, :], xt[:, 2880:3240],
                                 start=False, stop=False)
            else:
                # last sample: diminishing chunks so only ~1us of fold
                # work trails the final byte
                nc.sync.dma_start(xt[:, 0:1440], x[b][:, 0:1440])
                nc.scalar.dma_start(xt[:, 1440:2880], x[b][:, 1440:2880])
                nc.sync.dma_start(xt[:, 2880:4320], x[b][:, 2880:4320])
                nc.scalar.dma_start(xt[:, 4320:5040], x[b][:, 4320:5040])
                nc.sync.dma_start(xt[:, 5040:5760], x[b][:, 5040:5760])
                # H0 tree on DVE as soon as its quarters land
                nc.vector.tensor_add(xt[:, 0:1440], xt[:, 0:1440],
                                     xt[:, 1440:2880])
                # Q2 folds on GPSIMD
                nc.gpsimd.tensor_tensor(xt[:, 2880:3600], xt[:, 2880:3600],
                                        xt[:, 3600:4320], ADD)
                nc.vector.tensor_add(xt[:, 0:720], xt[:, 0:720],
                                     xt[:, 720:1440])
                nc.vector.tensor_add(xt[:, 0:360], xt[:, 0:360],
                                     xt[:, 360:720])
                nc.tensor.matmul(ps[:], O[:, b, :], xt[:, 0:360],
                                 start=False, stop=False)
                nc.gpsimd.tensor_tensor(xt[:, 2880:3240], xt[:, 2880:3240],
                                        xt[:, 3240:3600], ADD)
                # E6 (slices 12,13) on DVE
                nc.vector.tensor_add(xt[:, 4320:4680], xt[:, 4320:4680],
                                     xt[:, 4680:5040])
                nc.vector.tensor_add(xt[:, 2880:3240], xt[:, 2880:3240],
                                     xt[:, 4320:4680])
                # E7 (slices 14,15) is the last chunk on the wire
                nc.vector.tensor_add(xt[:, 5040:5400], xt[:, 5040:5400],
                                     xt[:, 5400:5760])
                nc.vector.tensor_add(xt[:, 2880:3240], xt[:, 2880:3240],
                                     xt[:, 5040:5400])
                nc.tensor.matmul(ps[:], O[:, b, :], xt[:, 2880:3240],
                                 start=False, stop=True)

        d16 = small.tile([BS, TH], F32)
        nc.vector.scalar_tensor_tensor(
            d16[:], ps[:], 1.0, t16,
            op0=mybir.AluOpType.mult, op1=mybir.AluOpType.subtract,
        )
        sq16 = small.tile([BS, TH], F32)
        nc.vector.scalar_tensor_tensor(
            sq16[:], d16[:], 1.0, d16[:],
            op0=mybir.AluOpType.mult, op1=mybir.AluOpType.mult,
        )
        sqw16 = small.tile([BS, TH], F32)
        red = small.tile([BS, 1], F32)
        nc.vector.scalar_tensor_tensor(
            sqw16[:], sq16[:], 1.0, w16,
            op0=mybir.AluOpType.mult, op1=mybir.AluOpType.mult,
            accum_out=red[:],
        )
        nc.sync.dma_start(out[:], red[:])
    nc.compile()
    return nc


def _target_and_weight(theta_min: np.ndarray, theta_max: np.ndarray):
    """Gaussian soft target T and distance weight w, [B, TH] float32 each.

    Mirrors the reference formulas (computed in float64, cast to float32;
    differences vs the f32 jax pipeline are O(1 ulp))."""
    theta = np.arange(TH, dtype=np.float64)[None, None, :]      # [1, 1, TH]
    tmin = theta_min.astype(np.float64)[:, :, None]             # [B, K, 1]
    tmax = theta_max.astype(np.float64)[:, :, None]

    center_wrap = np.mod(0.5 * (tmin + tmax + 360.0), 360.0)
    center_t = np.where(tmin <= tmax, 0.5 * (tmin + tmax), center_wrap)
    d = np.abs(theta - center_t)
    dist_t = np.minimum(d, 360.0 - d)                           # [B, K, TH]
    T = np.clip(np.exp(-0.5 * (dist_t / SIGMA) ** 2).sum(axis=1), 0.0, 1.0)

    center_w = (tmin + np.mod(tmax - tmin, 360.0)) / 2.0
    dw = np.abs(theta - center_w)
    dist_w = np.minimum(dw, 360.0 - dw)
    w = 1.0 + ALPHA_WEIGHT * (dist_w.max(axis=1) / 180.0)       # [B, TH]

    # Feed the device T' = R*T and w' = w/R^2 (both exact scalings by
    # powers of two) so it can use the raw radial sums S instead of the
    # mean A = S/R:  ((S - R*T)^2 * w/R^2) == ((A - T)^2 * w).
    Tp = (T * np.float32(R)).astype(np.float32)
    wp = (w / np.float32(R) ** 2).astype(np.float32)
    return Tp, wp


_NC_CACHE = None


def _get_nc():
    global _NC_CACHE
    if _NC_CACHE is None:
        _NC_CACHE = _build_nc()
    return _NC_CACHE


def _run(mask_pred, theta_min, theta_max, trace=False, trace_kwargs=None,
         trace_cores=None):
    mask_pred = np.asarray(mask_pred, dtype=np.float32)
    theta_min = np.asarray(theta_min)
    theta_max = np.asarray(theta_max)
    T, w = _target_and_weight(theta_min, theta_max)

    in_maps = []
    for i in range(N_CORES):
        sl = slice(i * BS, (i + 1) * BS)
        x_core = np.ascontiguousarray(mask_pred[sl, 0]).reshape(BS, 128, Q * TH)
        tw_core = np.stack([T[sl], w[sl]])
        in_maps.append({"x": x_core, "tw": tw_core})

    kwargs = {}
    if trace:
        kwargs["trace"] = True
        if trace_kwargs:
            kwargs["trace_kwargs"] = trace_kwargs
        if trace_cores is not None:
            kwargs["trace_cores"] = trace_cores
    res = run_bass_kernel_spmd(_get_nc(), in_maps, core_ids=list(range(N_CORES)),
                               **kwargs)
    per_sample = np.concatenate(
        [res.results[i]["out"][:, 0] for i in range(N_CORES)]
    )
    total = per_sample.astype(np.float64).sum() / (TH * B)
    return np.float32(LAMBDA_ANG * total), res


def kernel(mask_pred: np.ndarray, theta_min: np.ndarray,
           theta_max: np.ndarray) -> np.ndarray:
    loss, _ = _run(mask_pred, theta_min, theta_max)
    return np.asarray(loss, dtype=np.float32)


# revision 5
# speedup vs baseline: 1.1484x; 1.1484x over previous
"""Trainium2 Bass kernel for CMELossAngularProfileMSE_V2.

Strategy (pure data parallel over batch, 8 NeuronCores):
  - Shard B=128 samples -> 16 per core.
  - Per core, per sample: DMA a [128, 5760] tile in 2 half-sample
    chunks on the sync queue (r-major within partition: partition p
    holds r in [16p, 16p+16), free dim = 16*360 contiguous). A single
    queue keeps the full ~26.5 GB/s/engine wire rate (a second queue
    splits the 16 DMA engines and loses ~15% aggregate) and makes H0
    land mid-sample so folds overlap H1's transfer.
  - The 16->1 q-slice fold is split across engines so no engine comes
    close to the ~6.3us/sample wire cadence (DVE alone at ~6.4us/sample
    was the old limiter and made the pipeline fragile):
      GPSIMD: H0 head 1440-add (~3.2us), runs while H1 streams
      DVE:    H1 tree 1440+720+360 then H0 tail 720+360 (~4.5us)
      PE:     2 one-hot matmuls accumulating the two surviving
              360-slices into PSUM row b (~3.4us incl fp32 split)
  - Host precomputes T' = R*T and w' = w/R^2 (exact power-of-two
    scalings of the Gaussian target / distance weight derived from
    theta_min/theta_max), so the device epilogue is just
    sum_theta((S - T')^2 * w') per sample -> out [16, 1], on DVE.
  - Host: loss = sum(all per-sample sums) / (360 * 128).
"""
import numpy as np

import concourse.bacc as bacc
import concourse.tile as tile
from concourse import mybir
from concourse.bass_utils import run_bass_kernel_spmd

F32 = mybir.dt.float32
ADD = mybir.AluOpType.add

N_CORES = 8
B = 128            # full batch
BS = B // N_CORES  # samples per core (16)
R = 2048
TH = 360
Q = 16             # r-slices per partition (2048 = 128 * 16)
SIGMA = 10.0
ALPHA_WEIGHT = 2.0
LAMBDA_ANG = 1.0

H = (Q // 2) * TH  # half-sample width (2880)


def _build_nc():
    nc = bacc.Bacc("TRN2", target_bir_lowering=False, debug=False)
    x = nc.dram_tensor("x", [BS, 128, Q * TH], F32, kind="ExternalInput").ap()
    tw = nc.dram_tensor("tw", [2, BS, TH], F32, kind="ExternalInput").ap()
    out = nc.dram_tensor("out", [BS, 1], F32, kind="ExternalOutput").ap()

    from contextlib import ExitStack
    with tile.TileContext(nc) as tc, ExitStack() as ctx:
        consts = ctx.enter_context(tc.tile_pool(name="consts", bufs=1))
        inp = ctx.enter_context(tc.tile_pool(name="inp", bufs=8))
        psum = ctx.enter_context(tc.tile_pool(name="psum", bufs=1, space="PSUM"))
        small = ctx.enter_context(tc.tile_pool(name="small", bufs=1))

        # one-hot weight matrices: O[:, b, j] = 1 if j == b else 0
        # (PSUM out base partition must be 0/32/64, so per-row matmuls
        # need the one-hot trick; built on gpsimd, idle at startup)
        O = consts.tile([128, BS, BS], F32)
        nc.gpsimd.memset(O[:], 0.0)
        for b in range(BS):
            nc.gpsimd.memset(O[:, b, b:b + 1], 1.0)

        t16w16 = small.tile([BS, 2, TH], F32)
        t16 = t16w16[:, 0, :]
        w16 = t16w16[:, 1, :]

        ps = psum.tile([BS, TH], F32)
        for b in range(BS):
            xt = inp.tile([128, Q * TH], F32)
            if b < BS - 1:
                # steady state: two half-sample chunks
                nc.sync.dma_start(xt[:, 0:H], x[b][:, 0:H])
                nc.sync.dma_start(xt[:, H:2 * H], x[b][:, H:2 * H])
                # H0 head fold on GPSIMD, overlapping H1's transfer
                nc.gpsimd.tensor_tensor(xt[:, 0:1440], xt[:, 0:1440],
                                        xt[:, 1440:2880], ADD)
                # H1 tree on DVE as soon as H1 lands
                nc.vector.tensor_add(xt[:, 2880:4320], xt[:, 2880:4320],
                                     xt[:, 4320:5760])
                nc.vector.tensor_add(xt[:, 2880:3600], xt[:, 2880:3600],
                                     xt[:, 3600:4320])
                nc.vector.tensor_add(xt[:, 2880:3240], xt[:, 2880:3240],
                                     xt[:, 3240:3600])
                nc.tensor.matmul(ps[:], O[:, b, :], xt[:, 2880:3240],
                                 start=(b == 0), stop=False)
                # H0 tail on DVE once GPSIMD's head add lands
                nc.vector.tensor_add(xt[:, 0:720], xt[:, 0:720],
                                     xt[:, 720:1440])
                nc.vector.tensor_add(xt[:, 0:360], xt[:, 0:360],
                                     xt[:, 360:720])
                nc.tensor.matmul(ps[:], O[:, b, :], xt[:, 0:360],
                                 start=False, stop=False)
            else:
                # last sample: diminishing chunks so only ~1us of fold
                # work trails the final byte
                nc.sync.dma_start(xt[:, 0:H], x[b][:, 0:H])
                nc.sync.dma_start(xt[:, 2880:4320], x[b][:, 2880:4320])
                nc.sync.dma_start(xt[:, 4320:5040], x[b][:, 4320:5040])
                nc.sync.dma_start(xt[:, 5040:5760], x[b][:, 5040:5760])
                # H0 head on GPSIMD while the rest streams
                nc.gpsimd.tensor_tensor(xt[:, 0:1440], xt[:, 0:1440],
                                        xt[:, 1440:2880], ADD)
                # H0 tail on DVE -> s0 at [0:360]
                nc.vector.tensor_add(xt[:, 0:720], xt[:, 0:720],
                                     xt[:, 720:1440])
                nc.vector.tensor_add(xt[:, 0:360], xt[:, 0:360],
                                     xt[:, 360:720])
                nc.tensor.matmul(ps[:], O[:, b, :], xt[:, 0:360],
                                 start=False, stop=False)
                # Q2 (slices 8-11) on DVE as it lands -> s1 at [2880:3240]
                nc.vector.tensor_add(xt[:, 2880:3600], xt[:, 2880:3600],
                                     xt[:, 3600:4320])
                nc.vector.tensor_add(xt[:, 2880:3240], xt[:, 2880:3240],
                                     xt[:, 3240:3600])
                # E6 (slices 12,13)
                nc.vector.tensor_add(xt[:, 4320:4680], xt[:, 4320:4680],
                                     xt[:, 4680:5040])
                nc.vector.tensor_add(xt[:, 2880:3240], xt[:, 2880:3240],
                                     xt[:, 4320:4680])
                # E7 (slices 14,15) is the last chunk on the wire
                nc.vector.tensor_add(xt[:, 5040:5400], xt[:, 5040:5400],
                                     xt[:, 5400:5760])
                nc.vector.tensor_add(xt[:, 2880:3240], xt[:, 2880:3240],
                                     xt[:, 5040:5400])
                nc.tensor.matmul(ps[:], O[:, b, :], xt[:, 2880:3240],
                                 start=False, stop=True)

        # tw holds T' = R*T and w' = w/R^2 (exact power-of-two scalings),
        # so the raw PSUM sums S feed the epilogue directly. Issued after
        # the bulk stream: it is only needed by the epilogue.
        nc.sync.dma_start(t16w16[:], tw.rearrange("two b t -> b two t"))

        d16 = small.tile([BS, TH], F32)
        nc.vector.scalar_tensor_tensor(
            d16[:], ps[:], 1.0, t16,
            op0=mybir.AluOpType.mult, op1=mybir.AluOpType.subtract,
        )
        sq16 = small.tile([BS, TH], F32)
        nc.vector.scalar_tensor_tensor(
            sq16[:], d16[:], 1.0, d16[:],
            op0=mybir.AluOpType.mult, op1=mybir.AluOpType.mult,
        )
        sqw16 = small.tile([BS, TH], F32)
        red = small.tile([BS, 1], F32)
        nc.vector.scalar_tensor_tensor(
            sqw16[:], sq16[:], 1.0, w16,
            op0=mybir.AluOpType.mult, op1=mybir.AluOpType.mult,
            accum_out=red[:],
        )
        nc.sync.dma_start(out[:], red[:])
    nc.compile()
    return nc


def _target_and_weight(theta_min: np.ndarray, theta_max: np.ndarray):
    """Gaussian soft target T and distance weight w, [B, TH] float32 each.

    Mirrors the reference formulas (computed in float64, cast to float32;
    differences vs the f32 jax pipeline are O(1 ulp))."""
    theta = np.arange(TH, dtype=np.float64)[None, None, :]      # [1, 1, TH]
    tmin = theta_min.astype(np.float64)[:, :, None]             # [B, K, 1]
    tmax = theta_max.astype(np.float64)[:, :, None]

    center_wrap = np.mod(0.5 * (tmin + tmax + 360.0), 360.0)
    center_t = np.where(tmin <= tmax, 0.5 * (tmin + tmax), center_wrap)
    d = np.abs(theta - center_t)
    dist_t = np.minimum(d, 360.0 - d)                           # [B, K, TH]
    T = np.clip(np.exp(-0.5 * (dist_t / SIGMA) ** 2).sum(axis=1), 0.0, 1.0)

    center_w = (tmin + np.mod(tmax - tmin, 360.0)) / 2.0
    dw = np.abs(theta - center_w)
    dist_w = np.minimum(dw, 360.0 - dw)
    w = 1.0 + ALPHA_WEIGHT * (dist_w.max(axis=1) / 180.0)       # [B, TH]

    # Feed the device T' = R*T and w' = w/R^2 (both exact scalings by
    # powers of two) so it can use the raw radial sums S instead of the
    # mean A = S/R:  ((S - R*T)^2 * w/R^2) == ((A - T)^2 * w).
    Tp = (T * np.float32(R)).astype(np.float32)
    wp = (w / np.float32(R) ** 2).astype(np.float32)
    return Tp, wp


_NC_CACHE = None


def _get_nc():
    global _NC_CACHE
    if _NC_CACHE is None:
        _NC_CACHE = _build_nc()
    return _NC_CACHE


def _run(mask_pred, theta_min, theta_max, trace=False, trace_kwargs=None,
         trace_cores=None):
    mask_pred = np.asarray(mask_pred, dtype=np.float32)
    theta_min = np.asarray(theta_min)
    theta_max = np.asarray(theta_max)
    T, w = _target_and_weight(theta_min, theta_max)

    in_maps = []
    for i in range(N_CORES):
        sl = slice(i * BS, (i + 1) * BS)
        x_core = np.ascontiguousarray(mask_pred[sl, 0]).reshape(BS, 128, Q * TH)
        tw_core = np.stack([T[sl], w[sl]])
        in_maps.append({"x": x_core, "tw": tw_core})

    kwargs = {}
    if trace:
        kwargs["trace"] = True
        if trace_kwargs:
            kwargs["trace_kwargs"] = trace_kwargs
        if trace_cores is not None:
            kwargs["trace_cores"] = trace_cores
    res = run_bass_kernel_spmd(_get_nc(), in_maps, core_ids=list(range(N_CORES)),
                               **kwargs)
    per_sample = np.concatenate(
        [res.results[i]["out"][:, 0] for i in range(N_CORES)]
    )
    total = per_sample.astype(np.float64).sum() / (TH * B)
    return np.float32(LAMBDA_ANG * total), res


def kernel(mask_pred: np.ndarray, theta_min: np.ndarray,
           theta_max: np.ndarray) -> np.ndarray:
    loss, _ = _run(mask_pred, theta_min, theta_max)
    return np.asarray(loss, dtype=np.float32)
